# revision 49
# baseline (speedup 1.0000x reference)
"""DIMKT recurrence kernel for Trainium2 (8 NeuronCores, batch-parallel).

Layout: state kept as [D=128 partitions, B_local free]. Per core B_local=32,
optionally split into independent batch streams for latency hiding.

Math per step t (per batch column b, all in [d, b] layout):
  isdf   = x_t - h_{t-1}
  preA1  = W_sdf1 @ isdf                      (psA[:, 0:BS])
  preA2  = 2*W_sdf2 @ isdf                    (psA[:, BS:2BS])
  uA     = sigmoid(psA)            -> sdf/2 = (uA2 - 0.5) * uA1
  preB1  = 2*W_pka1s @ sdf_half + p1_t        (psB[:, 0:BS])
  preB2  = 4*W_pka2s @ sdf_half + p2_t        (psB[:, BS:2BS])
  preC   = -W_kih   @ h_{t-1}   + kip_t       (psB[:, 2BS:3BS])
  uB     = sigmoid(psB)            -> pka/2 = (uB2 - 0.5) * uB1 ; gN = uB3
  d      = 2*pka_half - h_{t-1}
  h_t    = h_{t-1} + gN * d
  m_t    = x_{t+1} * h_t           -> y_t = sigmoid(ones^T @ m_t)

p1_t = W_pka1c@ct_t + b_pka1 ; p2_t = 2*(W_pka2c@ct_t + b_pka2)
kip_t = -(W_kic@ct_t + W_kiq@qd_t + W_kicd@cd_t + b_ki)
using tanh(v) = 2*sigmoid(2v) - 1 and 1 - sigmoid(L) = sigmoid(-L).
"""

import os
import sys

import numpy as np

for _p in ("/opt/trn_rl_repo",):
    if _p not in sys.path:
        sys.path.insert(0, _p)

import ml_dtypes  # noqa: E402

import concourse.bass as bass  # noqa: E402
import concourse.tile as tile  # noqa: E402
from concourse import bacc, mybir  # noqa: E402
from concourse.bass_utils import run_bass_kernel_spmd  # noqa: E402

F32 = mybir.dt.float32
F16 = mybir.dt.float16
BF16 = mybir.dt.bfloat16

AF = mybir.ActivationFunctionType
ALU = mybir.AluOpType

B, S, D = 256, 500, 128
NCORES = 8
BL = B // NCORES                     # 32 batch per core
T = S - 1                            # 499 recurrence steps
COLS = S * BL                        # 16000 x columns, col = t*BL + b
PCOLS = T * BL                       # 15968 partial columns
CHUNK = 512                          # phase-1 column chunk (PSUM bank)
YGRP = 16                            # recurrence steps per y matmul group
NYG = (T + YGRP - 1) // YGRP         # 32 groups (last = 3 steps)

# Tunables (env-overridable for experiments)
NSTREAMS = int(os.environ.get("DIMKT_STREAMS", "2"))
T_STEPS = int(os.environ.get("DIMKT_T", str(T)))   # reduced-T for sim debug
DT_LOOP = {"f16": F16, "bf16": BF16}[os.environ.get("DIMKT_LOOP_DT", "f16")]
LOOP_NP = {F16: np.float16, BF16: ml_dtypes.bfloat16}[DT_LOOP]

BS = BL // NSTREAMS                  # batch cols per stream
P1_ACT = os.environ.get("DIMKT_P1_ACT", "1") != "0"   # phase-1 casts on Act
PREFETCH = os.environ.get("DIMKT_PREFETCH", "0") != "0"  # DMA a window ahead
DMA_SPLIT = os.environ.get("DIMKT_DMA_SPLIT", "1") != "0"  # qd/cd on gpsimd ring
# engine choice per loop elementwise op: d=DVE, p=Pool/gpsimd
# NOTE: scalar_tensor_tensor does not compile on gpsimd (neuronxcc rejects
# it); positions 0-3 must stay "d" unless Q_TT rewrites q as tensor_mul.
ENG_SEL = os.environ.get("DIMKT_ENG", "ddddd")  # [sdfh, P, pkah, q, hn]
# q = pkah * gN2 (tensor_mul on Pool, gN2 = 2*gN precomputed off-chain)
Q_TT = os.environ.get("DIMKT_QTT", "0") != "0"
MG_POOL = os.environ.get("DIMKT_MG", "d") == "p"  # y-path mults on Pool
# per-stream override: "" off; e.g. "dp" = stream0 all-DVE, stream1 all-Pool
ENG_STREAM = os.environ.get("DIMKT_ENG_STREAM", "")
# move the y sigmoid to the host; device just copies psY -> SBUF on DVE
Y_HOST = os.environ.get("DIMKT_YHOST", "0") != "0"
# DMA the m = x*h tiles to DRAM; host does the d-reduction and sigmoid
# (drops the psY matmul and the ys PSUM->SBUF copy from the loop)
Y_DMA = os.environ.get("DIMKT_YDMA", "1") != "0"
# fuse pkah+q into one tensor_tensor_scan (4 phases/column, constants
# pre-written; q lands at stride-4 columns of the scan output)
SCAN = os.environ.get("DIMKT_SCAN", "1") != "0"
# v8: phase-1 pre-activations accumulate directly in PSUM chunk blocks
# (8 steps/chunk, 3 banks, double buffered); loop matmuls accumulate on
# top; no identity matmuls, no a123/p12 casts. Forces YGRP=8, SCAN, Y_DMA.
V8 = os.environ.get("DIMKT_V8", "0") != "0"
V8_SPC = int(os.environ.get("DIMKT_V8_SPC", "6"))   # steps per psum chunk
if V8:
    YGRP = V8_SPC
    NYG = (T + YGRP - 1) // YGRP
    SCAN = True
    Y_DMA = True


def build_program(n_steps=None, nstreams=None, has_sdf_bias=False):
    """Trace the Bass/Tile program for one core (SPMD across 8).

    v6 structure: gamma folded into round 1 (psA = [a1|a2|kin] + W@h terms,
    gamma = sigmoid(-preC) via a second scale=-1 activation), tail shortened
    to pkah -> q on the chain (hn = P + q off-chain, P = gamma*h), h-matmuls
    split into W@P (early) + W@q (late) parts, phase-1 elementwise on the
    otherwise-idle gpsimd, and input DMA split over the SP and gpsimd queues.
    """
    n_steps = T_STEPS if n_steps is None else n_steps
    ns = NSTREAMS if nstreams is None else nstreams
    bs = BL // ns

    nc = bacc.Bacc(
        "TRN2", target_bir_lowering=False, debug=False, num_devices=NCORES
    )

    def _eng(i, s=0):
        if ENG_STREAM:
            sel = ENG_STREAM[s % len(ENG_STREAM)]
        else:
            sel = ENG_SEL[i]
        return nc.gpsimd if sel == "p" else nc.vector

    # ---- DRAM I/O ----
    emb = {
        name: nc.dram_tensor(name, [D, COLS], BF16, kind="ExternalInput").ap()
        for name in ("qe", "ce", "qd", "cd", "ct")
    }
    h0T = nc.dram_tensor("h0T", [D, BL], F32, kind="ExternalInput").ap()
    wpack = nc.dram_tensor("wpack", [D, 9 * D], BF16, kind="ExternalInput").ap()
    wloop = nc.dram_tensor("wloop", [D, 8 * D], DT_LOOP, kind="ExternalInput").ap()
    bpack = nc.dram_tensor("bpack", [D, 6], F32, kind="ExternalInput").ap()
    idf16 = nc.dram_tensor("idf16", [D, D], DT_LOOP, kind="ExternalInput").ap()
    onesc = nc.dram_tensor("onesc", [D, 1], DT_LOOP, kind="ExternalInput").ap()
    if Y_DMA:
        ydram = None
        mdram = nc.dram_tensor(
            "m", [ns * NYG * D, YGRP * (BL // ns)], DT_LOOP,
            kind="ExternalOutput",
        ).ap()
    else:
        ydram = nc.dram_tensor(
            "y", [ns, NYG * YGRP * (BL // ns)], F32, kind="ExternalOutput"
        ).ap()

    with tile.TileContext(nc) as tc:
        import contextlib

        ctx = contextlib.ExitStack()
        with ctx:
            const = ctx.enter_context(tc.tile_pool(name="const", bufs=1))
            data = ctx.enter_context(tc.tile_pool(name="data", bufs=4))
            ld = ctx.enter_context(tc.tile_pool(name="ld", bufs=3))
            ps1 = ctx.enter_context(tc.tile_pool(name="ps1", bufs=2, space="PSUM"))
            work = ctx.enter_context(tc.tile_pool(name="work", bufs=4))
            psA_pool = ctx.enter_context(tc.tile_pool(name="psA", bufs=1, space="PSUM"))
            psB_pool = ctx.enter_context(tc.tile_pool(name="psB", bufs=1, space="PSUM"))
            psY_pool = ctx.enter_context(tc.tile_pool(name="psY", bufs=1, space="PSUM"))
            hpool = ctx.enter_context(tc.tile_pool(name="h", bufs=4))
            mpool = ctx.enter_context(tc.tile_pool(name="m", bufs=2))
            ypool = ctx.enter_context(tc.tile_pool(name="ys", bufs=2))

            # ---- constants ----
            wsb = const.tile([D, 9 * D], BF16)
            nc.sync.dma_start(wsb[:], wpack)
            wl = const.tile([D, 8 * D], DT_LOOP)
            nc.sync.dma_start(wl[:], wloop)
            bsb = const.tile([D, 6], F32)
            nc.sync.dma_start(bsb[:], bpack)
            idsb = const.tile([D, D], DT_LOOP)
            nc.sync.dma_start(idsb[:], idf16)
            onessb = const.tile([D, 1], DT_LOOP)
            nc.sync.dma_start(onessb[:], onesc)
            h0sb = const.tile([D, BL], F32)
            nc.sync.dma_start(h0sb[:], h0T)

            bx = bsb[:, 0:1]
            b_p1 = bsb[:, 1:2]
            b_p2 = bsb[:, 2:3]
            b_kin = bsb[:, 3:4]
            b_s1 = bsb[:, 4:5]
            b_s2x2 = bsb[:, 5:6]

            # ---- big SBUF arrays: per-chunk tiles (16 steps each) ----
            xcols = S * BL if n_steps == T else (n_steps + 1) * BL
            pcols = n_steps * BL
            nck = (max(xcols, pcols) + CHUNK - 1) // CHUNK
            spc = CHUNK // BL                        # steps per chunk (16)

            xtiles = {}
            p12views = {}
            a123views = {}

            ld_tiles = {}

            def emit_dma(k):
                """Issue the input loads for chunk k (one window ahead)."""
                x0 = k * CHUNK
                xn = min(CHUNK, xcols - x0)
                pn = min(CHUNK, pcols - x0)
                et = {}
                need = []
                if xn > 0:
                    need += ["qe", "ce", "qd", "cd"]
                if pn > 0:
                    need += ["ct"] + (["qd", "cd"] if xn <= 0 else [])
                ncols = max(xn, pn)
                for name in dict.fromkeys(need):
                    et[name] = ld.tile(
                        [D, CHUNK], BF16, tag=f"ld_{name}", name=f"ld_{name}"
                    )
                    # split input streaming over two DMA queues: qd/cd on the
                    # gpsimd software-DGE ring, the rest on the SP HWDGE ring
                    eng = (
                        nc.gpsimd
                        if (DMA_SPLIT and name in ("qd", "cd"))
                        else nc.sync
                    )
                    eng.dma_start(
                        et[name][:, 0:ncols], emb[name][:, x0 : x0 + ncols]
                    )
                ld_tiles[k] = et

            def emit_phase1_piece(k, piece):
                """One slice of chunk k's precompute (spread across steps)."""
                x0 = k * CHUNK
                xn = min(CHUNK, xcols - x0)
                pn = min(CHUNK, pcols - x0)
                et = ld_tiles[k]
                ntt = pn // BL
                if piece == 0:
                    if xn <= 0:
                        return
                    xtiles[k] = data.tile(
                        [D, CHUNK], DT_LOOP, tag="xc", name=f"x{k}"
                    )
                    psX = ps1.tile([D, CHUNK], F32, tag="ps1")
                    for c, nm in enumerate(("qe", "ce", "qd", "cd")):
                        nc.tensor.matmul(
                            psX[:, 0:xn],
                            wsb[:, 128 * c : 128 * (c + 1)],
                            et[nm][:, 0:xn],
                            start=(c == 0),
                            stop=(c == 3),
                        )
                    nc.vector.tensor_scalar(
                        xtiles[k][:, 0:xn], psX[:, 0:xn], bx, None, ALU.add
                    )
                elif piece == 1:
                    if pn <= 0:
                        return
                    a123 = data.tile(
                        [D, 3 * CHUNK], DT_LOOP, tag="a123", name=f"a123_{k}"
                    )
                    a123views[k] = a123[:].rearrange(
                        "p (t g b) -> p t g b", g=3, b=BL
                    )
                    psQ1 = ps1.tile([D, CHUNK], F32, tag="ps1")
                    nc.tensor.matmul(
                        psQ1[:, 0:pn], wl[:, 0:128], xtiles[k][:, 0:pn],
                        start=True, stop=True,
                    )
                    q1v = psQ1[:, 0:pn].rearrange("p (t b) -> p t b", b=BL)
                    nc.vector.tensor_scalar(
                        a123views[k][:, 0:ntt, 0, :], q1v, -1.0, b_s1,
                        ALU.mult, ALU.add,
                    )
                elif piece == 2:
                    if pn <= 0:
                        return
                    psQ2 = ps1.tile([D, CHUNK], F32, tag="ps1")
                    nc.tensor.matmul(
                        psQ2[:, 0:pn], wl[:, 128:256], xtiles[k][:, 0:pn],
                        start=True, stop=True,
                    )
                    q2v = psQ2[:, 0:pn].rearrange("p (t b) -> p t b", b=BL)
                    nc.vector.tensor_scalar(
                        a123views[k][:, 0:ntt, 1, :], q2v, -1.0, b_s2x2,
                        ALU.mult, ALU.add,
                    )
                elif piece == 3:
                    if pn <= 0:
                        return
                    psK = ps1.tile([D, CHUNK], F32, tag="ps1")
                    nc.tensor.matmul(
                        psK[:, 0:pn], wsb[:, 768:896], et["ct"][:, 0:pn],
                        start=True, stop=False,
                    )
                    nc.tensor.matmul(
                        psK[:, 0:pn], wsb[:, 896:1024], et["qd"][:, 0:pn],
                        start=False, stop=False,
                    )
                    nc.tensor.matmul(
                        psK[:, 0:pn], wsb[:, 1024:1152], et["cd"][:, 0:pn],
                        start=False, stop=True,
                    )
                    kv = psK[:, 0:pn].rearrange("p (t b) -> p t b", b=BL)
                    if P1_ACT:
                        nc.scalar.activation(
                            a123views[k][:, 0:ntt, 2, :], kv, AF.Identity,
                            bias=b_kin,
                        )
                    else:
                        nc.vector.tensor_scalar(
                            a123views[k][:, 0:ntt, 2, :], kv, b_kin, None,
                            ALU.add,
                        )
                elif piece == 4:
                    if pn <= 0:
                        return
                    p12 = data.tile(
                        [D, 2 * CHUNK], DT_LOOP, tag="p12", name=f"p12_{k}"
                    )
                    p12views[k] = p12[:].rearrange(
                        "p (t g b) -> p t g b", g=2, b=BL
                    )
                    psP1 = ps1.tile([D, CHUNK], F32, tag="ps1")
                    nc.tensor.matmul(
                        psP1[:, 0:pn], wsb[:, 512:640], et["ct"][:, 0:pn],
                        start=True, stop=True,
                    )
                    p1v = psP1[:, 0:pn].rearrange("p (t b) -> p t b", b=BL)
                    if P1_ACT:
                        nc.scalar.activation(
                            p12views[k][:, 0:ntt, 0, :], p1v, AF.Identity,
                            bias=b_p1,
                        )
                    else:
                        nc.vector.tensor_scalar(
                            p12views[k][:, 0:ntt, 0, :], p1v, b_p1, None,
                            ALU.add,
                        )
                elif piece == 5:
                    if pn <= 0:
                        return
                    psP2 = ps1.tile([D, CHUNK], F32, tag="ps1")
                    nc.tensor.matmul(
                        psP2[:, 0:pn], wsb[:, 640:768], et["ct"][:, 0:pn],
                        start=True, stop=True,
                    )
                    p2v = psP2[:, 0:pn].rearrange("p (t b) -> p t b", b=BL)
                    if P1_ACT:
                        nc.scalar.activation(
                            p12views[k][:, 0:ntt, 1, :], p2v, AF.Identity,
                            bias=b_p2,
                        )
                    else:
                        nc.vector.tensor_scalar(
                            p12views[k][:, 0:ntt, 1, :], p2v, b_p2, None,
                            ALU.add,
                        )

            # h init: cast h0 to loop dtype
            hprev = []
            for s in range(ns):
                h0c = hpool.tile([D, bs], DT_LOOP, tag=f"h0{s}", name=f"h0c{s}")
                nc.vector.tensor_copy(h0c[:], h0sb[:, s * bs : (s + 1) * bs])
                hprev.append(h0c)

            W1 = wl[:, 0:128]
            W2 = wl[:, 128:256]
            Wp1 = wl[:, 256:384]
            Wp2 = wl[:, 384:512]
            Wkh = wl[:, 512:640]
            W1p = wl[:, 640:768]      # +W1.T   (for the negP operand)
            W2p = wl[:, 768:896]      # +2W2.T
            Wkhp = wl[:, 896:1024]    # +Wki1.T

            hgrp = [None] * ns
            sdfh_cur = [None] * ns
            uA_cur = [None] * ns
            P_cur = [None] * ns
            q_cur = [None] * ns
            gN2_cur = [None] * ns

            # scan-fused H2 tail: per-stream persistent buffers.
            # layout (stride 4 per batch col b):
            #   d1 = sc[:, 0:4bs]    cols 4b+{0,1,2,3} = [uB1, gN2, 0, 1]
            #   d0 = sc[:, 4bs:8bs]  cols 4b+{0,1,2,3} = [uB2, 0, 0, -0.5]
            # scan (init -0.5): s0=(uB2-.5)*uB1=pkah; s1=pkah*gN2=q;
            #                   s2=0; s3=-0.5 (re-init for next column)
            scbuf = []
            qsc = []
            if SCAN:
                scpool = ctx.enter_context(tc.tile_pool(name="scan", bufs=1))
                for s in range(ns):
                    sc = scpool.tile([D, 8 * bs], DT_LOOP, tag=f"sc{s}", name=f"sc{s}")
                    d1v = sc[:, 0 : 4 * bs].rearrange("p (b f) -> p b f", f=4)
                    d0v = sc[:, 4 * bs : 8 * bs].rearrange(
                        "p (b f) -> p b f", f=4
                    )
                    nc.vector.memset(d1v[:, :, 2:3], 0.0)
                    nc.vector.memset(d1v[:, :, 3:4], 1.0)
                    nc.vector.memset(d0v[:, :, 1:3], 0.0)
                    nc.vector.memset(d0v[:, :, 3:4], -0.5)
                    scbuf.append(sc)
                    qs = scpool.tile([D, 4 * bs], DT_LOOP, tag=f"qsc{s}")
                    qsc.append(qs)

            def emit_H1(s, t):
                """Round 1 of step t: psA = [a1|a2|kin] - [W1;2W2;Wki]@h,
                uA = sigmoid(psA) (-> uA1, uA2, gN), gamma = sigmoid(-preC),
                sdfh, P = gamma*h."""
                g = t % YGRP
                h = hprev[s]
                if g == 0:
                    hgrp[s] = hpool.tile(
                        [D, YGRP * bs], DT_LOOP, tag=f"hg{s}", name=f"hg{s}",
                        bufs=2,
                    )
                psA = psA_pool.tile([D, 3 * bs], F32, tag=f"psA{s}", name="psA")
                nc.tensor.matmul(
                    psA[:, 0 : 3 * bs], idsb[:],
                    a123views[t // spc][:, t % spc, :, s * bs : (s + 1) * bs],
                    start=True, stop=False,
                )
                if t == 0:
                    for c, W in enumerate((W1, W2, Wkh)):
                        nc.tensor.matmul(
                            psA[:, c * bs : (c + 1) * bs], W, h[:],
                            start=False, stop=(c == 2),
                        )
                else:
                    # h_{t-1} = q - negP: early +W@negP part, late -W@q part
                    # (W1/W2/Wkh slots hold negated weights; W*p positive)
                    for c, W in enumerate((W1p, W2p, Wkhp)):
                        nc.tensor.matmul(
                            psA[:, c * bs : (c + 1) * bs], W, P_cur[s][:],
                            start=False, stop=False,
                        )
                    for c, W in enumerate((W1, W2, Wkh)):
                        nc.tensor.matmul(
                            psA[:, c * bs : (c + 1) * bs], W, q_cur[s][:],
                            start=False, stop=(c == 2),
                        )
                uA = work.tile([D, 3 * bs], DT_LOOP, tag=f"uA{s}", name="uA")
                nc.scalar.activation(uA[:], psA[:], AF.Sigmoid)
                sdfh = work.tile([D, bs], DT_LOOP, tag=f"sdfh{s}", name="sdfh")
                _eng(0, s).scalar_tensor_tensor(
                    sdfh[:], uA[:, bs : 2 * bs], -0.5, uA[:, 0:bs],
                    ALU.add, ALU.mult,
                )
                # negP = (gN - 1) * h = -(1-gN)*h;  h_t = q_t - negP_t
                P = work.tile([D, bs], DT_LOOP, tag=f"P{s}", name="P")
                _eng(1, s).scalar_tensor_tensor(
                    P[:], uA[:, 2 * bs : 3 * bs], -1.0, h[:],
                    ALU.add, ALU.mult,
                )
                if Q_TT:
                    gN2 = work.tile([D, bs], DT_LOOP, tag=f"gN2{s}", name="gN2")
                    nc.gpsimd.tensor_scalar(
                        gN2[:], uA[:, 2 * bs : 3 * bs], 2.0, None, ALU.mult
                    )
                    gN2_cur[s] = gN2
                if SCAN:
                    d1v = scbuf[s][:, 0 : 4 * bs].rearrange(
                        "p (b f) -> p b f", f=4
                    )
                    nc.vector.tensor_scalar(
                        d1v[:, :, 1:2],
                        uA[:, 2 * bs : 3 * bs].rearrange("p (b o) -> p b o", o=1),
                        2.0, None, ALU.mult,
                    )
                uA_cur[s] = uA
                sdfh_cur[s] = sdfh
                P_cur[s] = P

            def emit_H2(s, t):
                """Round 2 of step t: psB = [p1|p2] + Wp@sdfh, uB, then
                q = pka*gN and hn = P + q."""
                g = t % YGRP
                sdfh = sdfh_cur[s]
                psB = psB_pool.tile([D, 2 * bs], F32, tag=f"psB{s}", name="psB")
                nc.tensor.matmul(
                    psB[:, 0 : 2 * bs], idsb[:],
                    p12views[t // spc][:, t % spc, :, s * bs : (s + 1) * bs],
                    start=True, stop=False,
                )
                nc.tensor.matmul(
                    psB[:, 0:bs], Wp1, sdfh[:], start=False, stop=False
                )
                nc.tensor.matmul(
                    psB[:, bs : 2 * bs], Wp2, sdfh[:], start=False, stop=True
                )
                if SCAN:
                    sc = scbuf[s]
                    # uB1 -> d1 cols 4b+0, uB2 -> d0 cols 4b+0 (one act,
                    # outer stride 4bs over the two halves, inner stride 4)
                    scv = sc[:].rearrange("p (g b f) -> p g b f", g=2, f=4)
                    nc.scalar.activation(
                        scv[:, :, :, 0:1],
                        psB[:, 0 : 2 * bs].rearrange(
                            "p (g b o) -> p g b o", g=2, o=1
                        ),
                        AF.Sigmoid,
                    )
                    nc.vector.tensor_tensor_scan(
                        qsc[s][:],
                        sc[:, 4 * bs : 8 * bs],
                        sc[:, 0 : 4 * bs],
                        -0.5,
                        ALU.add,
                        ALU.mult,
                    )
                    q = qsc[s][:].rearrange("p (b f) -> p b f", f=4)[
                        :, :, 1:2
                    ].rearrange("p b o -> p (b o)")
                else:
                    uB = work.tile([D, 2 * bs], DT_LOOP, tag=f"uB{s}", name="uB")
                    nc.scalar.activation(uB[:], psB[:], AF.Sigmoid)
                    pkah = work.tile(
                        [D, bs], DT_LOOP, tag=f"pkah{s}", name="pkah"
                    )
                    _eng(2, s).scalar_tensor_tensor(
                        pkah[:], uB[:, bs : 2 * bs], -0.5, uB[:, 0:bs],
                        ALU.add, ALU.mult,
                    )
                    q = work.tile([D, bs], DT_LOOP, tag=f"q{s}", name="q")
                    if Q_TT:
                        nc.gpsimd.tensor_mul(q[:], pkah[:], gN2_cur[s][:])
                    else:
                        _eng(3, s).scalar_tensor_tensor(
                            q[:], pkah[:], 2.0, uA_cur[s][:, 2 * bs : 3 * bs],
                            ALU.mult, ALU.mult,
                        )
                    q = q[:]
                hn = hgrp[s][:, g * bs : (g + 1) * bs]
                _eng(4, s).tensor_sub(hn, q, P_cur[s][:])
                q_cur[s] = q
                hprev[s] = hn
                if g == YGRP - 1 or t == n_steps - 1:
                    emit_stream_y(s, t // YGRP, g + 1)

            def emit_stream_y(s, gi, gn):
                """y_t = sigmoid(x_{t+1} . h_t) for steps t in [16*gi, +gn).

                x step (16*gi + j + 1) for slot j: slots 0..14 live in chunk
                gi (local steps 1..15); slot 15 is chunk gi+1 local step 0.
                """
                mg = mpool.tile(
                    [D, YGRP * bs], DT_LOOP, tag=f"mg{s}", name=f"mg{s}"
                )
                hv = hgrp[s][:].rearrange("p (t b) -> p t b", b=bs)
                mv = mg[:].rearrange("p (t b) -> p t b", b=bs)
                xv = xtiles[gi][:].rearrange("p (t b) -> p t b", b=BL)
                n1 = min(gn, YGRP - 1)
                _mge = nc.gpsimd if MG_POOL else nc.vector
                _mge.tensor_mul(
                    mv[:, 0:n1, :],
                    xv[:, 1 : n1 + 1, s * bs : (s + 1) * bs],
                    hv[:, 0:n1, :],
                )
                if gn == YGRP:
                    _mge.tensor_mul(
                        mv[:, YGRP - 1, :],
                        xtiles[gi + 1][:, s * bs : (s + 1) * bs],
                        hv[:, YGRP - 1, :],
                    )
                if Y_DMA:
                    r0 = (s * NYG + gi) * D
                    nc.sync.dma_start(mdram[r0 : r0 + D, :], mg[:])
                    return
                psY = psY_pool.tile(
                    [1, YGRP * bs], F32, tag="psY", name="psY", bufs=2
                )
                nc.tensor.matmul(
                    psY[:, 0 : gn * bs], onessb[:], mg[:, 0 : gn * bs],
                    start=True, stop=True,
                )
                ys = ypool.tile([1, YGRP * bs], F32, tag=f"ys{s}", name="ys")
                if Y_HOST:
                    nc.vector.tensor_copy(ys[:, 0 : gn * bs], psY[:, 0 : gn * bs])
                else:
                    nc.scalar.activation(
                        ys[:, 0 : gn * bs], psY[:, 0 : gn * bs], AF.Sigmoid
                    )
                nc.sync.dma_start(
                    ydram[s : s + 1, gi * YGRP * bs : gi * YGRP * bs + gn * bs],
                    ys[:, 0 : gn * bs],
                )

            # ---- software-pipelined emission: phase1 chunk k, then steps
            # of chunk k-1. Streams run phase-offset: stream j leads stream
            # (j + ns//2) % ns by half a step, so while one group is in its
            # DVE tail phase the other is in its activation phase.
            half = ns // 2

            def emit_step(t):
                for j in range(ns):
                    emit_H1(j, t)
                    if ns == 1:
                        emit_H2(j, t)
                    else:
                        s2 = (j + half) % ns
                        t2 = t - (1 if j < half else 0)
                        if 0 <= t2 < n_steps:
                            emit_H2(s2, t2)

            # emission: chunk k's DMA + precompute lump, then steps of
            # chunk k-1. With DIMKT_PREFETCH=1 the DMA is issued one full
            # window ahead of the chunk's precompute.
            if PREFETCH:
                emit_dma(0)
                for k in range(nck + 1):
                    if k + 1 < nck:
                        emit_dma(k + 1)
                    if k < nck:
                        for j in range(6):
                            emit_phase1_piece(k, j)
                    if k >= 1:
                        for t in range(spc * (k - 1), min(spc * k, n_steps)):
                            emit_step(t)
            else:
                # spread chunk k's phase-1 pieces across chunk k-1's step
                # window (one piece every other step) so their PE/Act/DVE
                # slabs land in per-step engine idle gaps instead of as one
                # contiguous block that stalls the recurrence chain.
                for k in range(nck + 1):
                    pend = list(range(6)) if k < nck else []
                    if k < nck:
                        emit_dma(k)
                    if k >= 1:
                        steps = range(spc * (k - 1), min(spc * k, n_steps))
                        for i, t in enumerate(steps):
                            emit_step(t)
                            if pend and i % 2 == 0:
                                emit_phase1_piece(k, pend.pop(0))
                    for j in pend:
                        emit_phase1_piece(k, j)
            # drain: trailing H2 halves for streams still one half behind
            if ns > 1:
                for j in range(half):
                    emit_H2((j + half) % ns, n_steps - 1)

    nc.compile()
    return nc


def build_program_v8(n_steps=None, nstreams=None, has_bias=False):
    """v8: phase-1 pre-activations live in PSUM chunk blocks.

    Per 8-step chunk one [D, 1536] f32 PSUM tile (3 banks, pool bufs=2):
      stream s block at s*640: [a1 | a2 | kin | p1 | p2] (128 cols each,
      step t local lt at col lt*16), x block at [1280:1536].
    Phase-1 matmuls write the input-dependent parts (start=True); the
    recurrence matmuls accumulate W@P / W@q / Wp@sdfh on top
    (start=False, stop=True on each block's last). The activations read
    strided [p, g, 16] PSUM views directly - no casts, no idsb matmuls.
    Biases, when nonzero, enter via an indicator matmul per chunk.
    """
    n_steps = T_STEPS if n_steps is None else n_steps
    ns = NSTREAMS if nstreams is None else nstreams
    bs = BL // ns
    assert ns == 2 and bs == 16, "v8 layout assumes 2 streams of 16"
    spc = V8_SPC                              # steps per psum chunk
    CH = spc * BL                             # x cols per chunk
    assert spc * 80 <= 512, "stream chunk block must fit one psum bank"

    nc = bacc.Bacc(
        "TRN2", target_bir_lowering=False, debug=False, num_devices=NCORES
    )

    emb = {
        name: nc.dram_tensor(name, [D, COLS], BF16, kind="ExternalInput").ap()
        for name in ("qe", "ce", "qd", "cd", "ct")
    }
    h0T = nc.dram_tensor("h0T", [D, BL], F32, kind="ExternalInput").ap()
    wpack = nc.dram_tensor("wpack", [D, 9 * D], BF16, kind="ExternalInput").ap()
    wloop = nc.dram_tensor("wloop", [D, 8 * D], DT_LOOP, kind="ExternalInput").ap()
    bpack = nc.dram_tensor("bpack", [D, 6], F32, kind="ExternalInput").ap()
    if has_bias:
        biasmm = nc.dram_tensor("biasmm", [D, D], BF16, kind="ExternalInput").ap()
        bind = nc.dram_tensor("bind", [D, spc * 80], BF16, kind="ExternalInput").ap()
    mdram = nc.dram_tensor(
        "m", [ns * NYG * D, YGRP * bs], DT_LOOP, kind="ExternalOutput"
    ).ap()

    with tile.TileContext(nc) as tc:
        import contextlib

        ctx = contextlib.ExitStack()
        with ctx:
            const = ctx.enter_context(tc.tile_pool(name="const", bufs=1))
            data = ctx.enter_context(tc.tile_pool(name="data", bufs=4))
            ld = ctx.enter_context(tc.tile_pool(name="ld", bufs=3))
            work = ctx.enter_context(tc.tile_pool(name="work", bufs=4))
            pch = ctx.enter_context(
                tc.tile_pool(name="pchunk", bufs=3, space="PSUM")
            )
            xps_pool = ctx.enter_context(
                tc.tile_pool(name="xps", bufs=2, space="PSUM")
            )
            hpool = ctx.enter_context(tc.tile_pool(name="h", bufs=4))
            mpool = ctx.enter_context(tc.tile_pool(name="m", bufs=2))
            scpool = ctx.enter_context(tc.tile_pool(name="scan", bufs=1))

            wsb = const.tile([D, 9 * D], BF16)
            nc.sync.dma_start(wsb[:], wpack)
            wl = const.tile([D, 8 * D], DT_LOOP)
            nc.sync.dma_start(wl[:], wloop)
            bsb = const.tile([D, 6], F32)
            nc.sync.dma_start(bsb[:], bpack)
            h0sb = const.tile([D, BL], F32)
            nc.sync.dma_start(h0sb[:], h0T)
            if has_bias:
                bmm = const.tile([D, D], BF16)
                nc.sync.dma_start(bmm[:], biasmm)
                bin_ = const.tile([D, spc * 80], BF16)
                nc.sync.dma_start(bin_[:], bind)

            bx = bsb[:, 0:1]

            xcols = S * BL if n_steps == T else (n_steps + 1) * BL
            pcols = n_steps * BL
            nck = (max(xcols, pcols) + CH - 1) // CH

            xtiles = {}
            pchunks = {}
            ld_tiles = {}

            def emit_dma(k):
                x0 = k * CH
                xn = min(CH, xcols - x0)
                pn = min(CH, pcols - x0)
                et = {}
                need = []
                if xn > 0:
                    need += ["qe", "ce", "qd", "cd"]
                if pn > 0:
                    need += ["ct"] + (["qd", "cd"] if xn <= 0 else [])
                ncols = max(xn, pn)
                for name in dict.fromkeys(need):
                    et[name] = ld.tile(
                        [D, CH], BF16, tag=f"ld_{name}", name=f"ld_{name}"
                    )
                    eng = (
                        nc.gpsimd
                        if (DMA_SPLIT and name in ("qd", "cd"))
                        else nc.sync
                    )
                    eng.dma_start(
                        et[name][:, 0:ncols], emb[name][:, x0 : x0 + ncols]
                    )
                ld_tiles[k] = et

            def emit_phase1(k):
                """Chunk k: x matmuls+cast, then per-stream block fills.

                One [D, spc*80] psum tile per stream (fits one bank): step
                lt at col 80*lt = [a1|a2|kin|p1|p2] 16 cols each. Strided
                phase-1 matmuls fill each group across steps in one instr.
                """
                x0 = k * CH
                xn = min(CH, xcols - x0)
                pn = min(CH, pcols - x0)
                et = ld_tiles[k]
                if xn > 0:
                    xp = xps_pool.tile([D, CH], F32, tag="xp", name=f"xp{k}")
                    for c, nm in enumerate(("qe", "ce", "qd", "cd")):
                        nc.tensor.matmul(
                            xp[:, 0:xn],
                            wsb[:, 128 * c : 128 * (c + 1)],
                            et[nm][:, 0:xn],
                            start=(c == 0), stop=(c == 3),
                        )
                    xt = data.tile([D, CH], DT_LOOP, tag="xc", name=f"x{k}")
                    xtiles[k] = xt
                    if P1_ACT:
                        nc.scalar.activation(
                            xt[:, 0:xn], xp[:, 0:xn], AF.Identity, bias=bx
                        )
                    else:
                        nc.vector.tensor_scalar(
                            xt[:, 0:xn], xp[:, 0:xn], bx, None, ALU.add
                        )
                if pn <= 0:
                    return
                ln = pn // BL                  # local steps with data
                pcs = []
                for s in range(ns):
                    pc = pch.tile(
                        [D, spc * 80], F32, tag=f"pc{s}", name=f"pc{s}_{k}"
                    )
                    pcs.append(pc)
                pchunks[k] = pcs
                first = not has_bias
                for s in range(ns):
                    pc = pcs[s]
                    if has_bias:
                        nc.tensor.matmul(
                            pc[:, 0 : 80 * ln], bmm[:], bin_[:, 0 : 80 * ln],
                            start=True, stop=False,
                        )

                    def gview(og):
                        return (
                            pc[:, 0 : 80 * spc]
                            .rearrange("p (t f) -> p t f", f=80)[
                                :, 0:ln, og : og + bs
                            ]
                        )

                    xv = (
                        xtiles[k][:]
                        .rearrange("p (t b) -> p t b", b=BL)[
                            :, 0:ln, s * bs : (s + 1) * bs
                        ]
                    )

                    def ev(nm):
                        return (
                            et[nm][:, 0:pn]
                            .rearrange("p (t b) -> p t b", b=BL)[
                                :, :, s * bs : (s + 1) * bs
                            ]
                        )

                    # a1 = +W1@x (+b_s1), a2 = +2W2@x (+b_s2x2): use the
                    # positive-sign weight slots (the loop's -W slots handle
                    # the -W@h part)
                    nc.tensor.matmul(
                        gview(0), wl[:, 640:768], xv, start=first, stop=False
                    )
                    nc.tensor.matmul(
                        gview(16), wl[:, 768:896], xv, start=first, stop=False
                    )
                    for c, nm in enumerate(("ct", "qd", "cd")):
                        nc.tensor.matmul(
                            gview(32),
                            wsb[:, 768 + 128 * c : 896 + 128 * c],
                            ev(nm),
                            start=(first and c == 0), stop=False,
                        )
                    nc.tensor.matmul(
                        gview(48), wsb[:, 512:640], ev("ct"),
                        start=first, stop=False,
                    )
                    nc.tensor.matmul(
                        gview(64), wsb[:, 640:768], ev("ct"),
                        start=first, stop=False,
                    )

            W1 = wl[:, 0:128]
            W2 = wl[:, 128:256]
            Wp1 = wl[:, 256:384]
            Wp2 = wl[:, 384:512]
            Wkh = wl[:, 512:640]
            W1p = wl[:, 640:768]
            W2p = wl[:, 768:896]
            Wkhp = wl[:, 896:1024]

            hprev = []
            for s in range(ns):
                h0c = hpool.tile([D, bs], DT_LOOP, tag=f"h0{s}", name=f"h0c{s}")
                nc.vector.tensor_copy(h0c[:], h0sb[:, s * bs : (s + 1) * bs])
                hprev.append(h0c)

            hgrp = [None] * ns
            sdfh_cur = [None] * ns
            uA_cur = [None] * ns
            P_cur = [None] * ns
            q_cur = [None] * ns

            scbuf = []
            qsc = []
            for s in range(ns):
                sc = scpool.tile([D, 8 * bs], DT_LOOP, tag=f"sc{s}", name=f"sc{s}")
                d1v = sc[:, 0 : 4 * bs].rearrange("p (b f) -> p b f", f=4)
                d0v = sc[:, 4 * bs : 8 * bs].rearrange("p (b f) -> p b f", f=4)
                nc.vector.memset(d1v[:, :, 2:3], 0.0)
                nc.vector.memset(d1v[:, :, 3:4], 1.0)
                nc.vector.memset(d0v[:, :, 1:3], 0.0)
                nc.vector.memset(d0v[:, :, 3:4], -0.5)
                scbuf.append(sc)
                qs = scpool.tile([D, 4 * bs], DT_LOOP, tag=f"qsc{s}", name=f"qsc{s}")
                qsc.append(qs)

            def blocks(s, t):
                pc = pchunks[t // spc][s]
                lt = t % spc
                b0 = lt * 80
                return pc, b0

            def emit_H1(s, t):
                g = t % YGRP
                h = hprev[s]
                if g == 0:
                    hgrp[s] = hpool.tile(
                        [D, YGRP * bs], DT_LOOP, tag=f"hg{s}", name=f"hg{s}",
                        bufs=2,
                    )
                pc, b0 = blocks(s, t)
                dests = (
                    pc[:, b0 : b0 + 16],
                    pc[:, b0 + 16 : b0 + 32],
                    pc[:, b0 + 32 : b0 + 48],
                )
                if t == 0:
                    for c, W in enumerate((W1, W2, Wkh)):
                        nc.tensor.matmul(
                            dests[c], W, h[:], start=False, stop=True
                        )
                else:
                    for c, W in enumerate((W1p, W2p, Wkhp)):
                        nc.tensor.matmul(
                            dests[c], W, P_cur[s][:], start=False, stop=False
                        )
                    for c, W in enumerate((W1, W2, Wkh)):
                        nc.tensor.matmul(
                            dests[c], W, q_cur[s], start=False, stop=True
                        )
                uA = work.tile([D, 3 * bs], DT_LOOP, tag=f"uA{s}", name="uA")
                nc.scalar.activation(uA[:], pc[:, b0 : b0 + 48], AF.Sigmoid)
                sdfh = work.tile([D, bs], DT_LOOP, tag=f"sdfh{s}", name="sdfh")
                nc.vector.scalar_tensor_tensor(
                    sdfh[:], uA[:, bs : 2 * bs], -0.5, uA[:, 0:bs],
                    ALU.add, ALU.mult,
                )
                P = work.tile([D, bs], DT_LOOP, tag=f"P{s}", name="P")
                nc.vector.scalar_tensor_tensor(
                    P[:], uA[:, 2 * bs : 3 * bs], -1.0, h[:],
                    ALU.add, ALU.mult,
                )
                d1v = scbuf[s][:, 0 : 4 * bs].rearrange("p (b f) -> p b f", f=4)
                nc.vector.tensor_scalar(
                    d1v[:, :, 1:2],
                    uA[:, 2 * bs : 3 * bs].rearrange("p (b o) -> p b o", o=1),
                    2.0, None, ALU.mult,
                )
                uA_cur[s] = uA
                sdfh_cur[s] = sdfh
                P_cur[s] = P

            def emit_H2(s, t):
                g = t % YGRP
                sdfh = sdfh_cur[s]
                pc, b0 = blocks(s, t)
                nc.tensor.matmul(
                    pc[:, b0 + 48 : b0 + 64], Wp1, sdfh[:],
                    start=False, stop=True,
                )
                nc.tensor.matmul(
                    pc[:, b0 + 64 : b0 + 80], Wp2, sdfh[:],
                    start=False, stop=True,
                )
                psBv = pc[:, b0 + 48 : b0 + 80].rearrange(
                    "p (g c) -> p g c", g=2
                )
                sc = scbuf[s]
                scv = sc[:].rearrange("p (g b f) -> p g b f", g=2, f=4)
                nc.scalar.activation(
                    scv[:, :, :, 0:1],
                    psBv.rearrange("p g (b o) -> p g b o", o=1),
                    AF.Sigmoid,
                )
                nc.vector.tensor_tensor_scan(
                    qsc[s][:], sc[:, 4 * bs : 8 * bs], sc[:, 0 : 4 * bs],
                    -0.5, ALU.add, ALU.mult,
                )
                q = qsc[s][:].rearrange("p (b f) -> p b f", f=4)[
                    :, :, 1:2
                ].rearrange("p b o -> p (b o)")
                hn = hgrp[s][:, g * bs : (g + 1) * bs]
                nc.vector.tensor_sub(hn, q, P_cur[s][:])
                q_cur[s] = q
                hprev[s] = hn
                if g == YGRP - 1 or t == n_steps - 1:
                    emit_stream_y(s, t // YGRP, g + 1)

            def emit_stream_y(s, gi, gn):
                mg = mpool.tile(
                    [D, YGRP * bs], DT_LOOP, tag=f"mg{s}", name=f"mg{s}"
                )
                hv = hgrp[s][:].rearrange("p (t b) -> p t b", b=bs)
                mv = mg[:].rearrange("p (t b) -> p t b", b=bs)
                xv = xtiles[gi][:].rearrange("p (t b) -> p t b", b=BL)
                n1 = min(gn, YGRP - 1)
                nc.vector.tensor_mul(
                    mv[:, 0:n1, :],
                    xv[:, 1 : n1 + 1, s * bs : (s + 1) * bs],
                    hv[:, 0:n1, :],
                )
                if gn == YGRP:
                    nc.vector.tensor_mul(
                        mv[:, YGRP - 1, :],
                        xtiles[gi + 1][:, s * bs : (s + 1) * bs],
                        hv[:, YGRP - 1, :],
                    )
                r0 = (s * NYG + gi) * D
                nc.sync.dma_start(mdram[r0 : r0 + D, :], mg[:])

            half = ns // 2

            def emit_step(t):
                for j in range(ns):
                    emit_H1(j, t)
                    s2 = (j + half) % ns
                    t2 = t - (1 if j < half else 0)
                    if 0 <= t2 < n_steps:
                        emit_H2(s2, t2)

            for k in range(nck + 1):
                if k < nck:
                    emit_dma(k)
                    emit_phase1(k)
                if k >= 1:
                    for t in range(spc * (k - 1), min(spc * k, n_steps)):
                        emit_step(t)
            for j in range(half):
                emit_H2((j + half) % ns, n_steps - 1)

    nc.compile()
    return nc


_CACHE = {}


def _get_program(has_sdf_bias):
    key = (T_STEPS, NSTREAMS, DT_LOOP, has_sdf_bias, V8)
    if key not in _CACHE:
        if V8:
            _CACHE[key] = build_program_v8(has_bias=has_sdf_bias)
        else:
            _CACHE[key] = build_program(has_sdf_bias=has_sdf_bias)
    return _CACHE[key]


def prep_core_inputs(inputs, core, has_sdf_bias):
    """Build the per-core input map (host-side shard + transpose + pack)."""
    sl = slice(core * BL, (core + 1) * BL)
    m = {}
    for key, name in (
        ("question_emb", "qe"),
        ("concept_emb", "ce"),
        ("question_diff_emb", "qd"),
        ("concept_diff_emb", "cd"),
        ("correctness_emb", "ct"),
    ):
        e = inputs[key][sl]                       # [BL, S, D]
        et = np.ascontiguousarray(e.transpose(2, 1, 0)).reshape(D, COLS)
        m[name] = et.astype(ml_dtypes.bfloat16)
    m["h0T"] = np.ascontiguousarray(inputs["h0"][sl].T).astype(np.float32)
    m.update(_weight_pack(inputs, has_sdf_bias))
    return m


def _weight_pack(inputs, has_sdf_bias):
    Wx = inputs["Wx"]            # [D, 4D]
    Wp1 = inputs["W_pka1"]       # [D, 2D]
    Wp2 = inputs["W_pka2"]
    Wki = inputs["W_ki"]         # [D, 4D]
    W1 = inputs["W_sdf1"]
    W2 = inputs["W_sdf2"]

    wpack = np.concatenate(
        [Wx[:, 128 * c : 128 * (c + 1)].T for c in range(4)]
        + [
            Wp1[:, 128:256].T,
            2.0 * Wp2[:, 128:256].T,
            -Wki[:, 128:256].T,
            -Wki[:, 256:384].T,
            -Wki[:, 384:512].T,
        ],
        axis=1,
    )
    wloop = np.concatenate(
        [
            -W1.T,
            -2.0 * W2.T,
            2.0 * Wp1[:, 0:128].T,
            4.0 * Wp2[:, 0:128].T,
            -Wki[:, 0:128].T,
            W1.T,
            2.0 * W2.T,
            Wki[:, 0:128].T,
        ],
        axis=1,
    )
    bpack = np.stack(
        [
            inputs["bx"],
            inputs["b_pka1"],
            2.0 * inputs["b_pka2"],
            -inputs["b_ki"],
            inputs["b_sdf1"],
            2.0 * inputs["b_sdf2"],
        ],
        axis=1,
    )
    out = {
        "wpack": np.ascontiguousarray(wpack).astype(ml_dtypes.bfloat16),
        "wloop": np.ascontiguousarray(wloop).astype(LOOP_NP),
        "bpack": np.ascontiguousarray(bpack).astype(np.float32),
        "idf16": np.eye(D, dtype=LOOP_NP),
        "onesc": np.ones((D, 1), dtype=LOOP_NP),
    }
    if V8 and has_sdf_bias:
        # indicator-matmul bias injection: psum[block g] += biasrow_g
        bmm = np.zeros((D, D), dtype=np.float32)
        bmm[0] = inputs["b_sdf1"]
        bmm[1] = 2.0 * inputs["b_sdf2"]
        bmm[2] = -inputs["b_ki"]
        bmm[3] = inputs["b_pka1"]
        bmm[4] = 2.0 * inputs["b_pka2"]
        ind = np.zeros((D, V8_SPC * 80), dtype=np.float32)
        cols = np.arange(V8_SPC * 80)
        for g in range(5):
            ind[g, (cols % 80) // 16 == g] = 1.0
        out["biasmm"] = bmm.astype(ml_dtypes.bfloat16)
        out["bind"] = ind.astype(ml_dtypes.bfloat16)
    return out


def decode_y(results, n_steps=None, nstreams=None):
    """[ns, NYG*YGRP*bs] per core -> full [B, T] float32."""
    n_steps = T_STEPS if n_steps is None else n_steps
    ns = NSTREAMS if nstreams is None else nstreams
    bs = BL // ns
    y = np.empty((B, n_steps), dtype=np.float32)
    if Y_DMA:
        for c, res in enumerate(results):
            md = np.asarray(res["m"], dtype=np.float32).reshape(
                ns, NYG, D, YGRP, bs
            )
            dots = md.sum(axis=2)                  # [ns, NYG, YGRP, bs]
            for s in range(ns):
                blk = dots[s].reshape(NYG * YGRP, bs)[:n_steps]  # [T, bs]
                y[c * BL + s * bs : c * BL + (s + 1) * bs, :] = blk.T
        return 1.0 / (1.0 + np.exp(-y))
    tt = np.arange(n_steps)
    col = (tt // YGRP) * (YGRP * bs) + (tt % YGRP) * bs
    for c, res in enumerate(results):
        yd = res["y"]                              # [ns, NYG*YGRP*bs]
        for s in range(ns):
            block = yd[s][col[:, None] + np.arange(bs)[None, :]]  # [T, bs]
            y[c * BL + s * bs : c * BL + (s + 1) * bs, :] = block.T
    if Y_HOST:
        y = 1.0 / (1.0 + np.exp(-y))
    return y


def timed_run(inputs, iters=10):
    """Run on 8 cores with executable reuse; returns (y, min_wall_ns).

    Mirrors bass2jax.run_bass_via_pjrt's multi-core path but keeps inputs
    on-device and times repeated executions (min over `iters`).
    """
    import time

    import jax
    from jax.sharding import Mesh, PartitionSpec
    from jax.experimental.shard_map import shard_map

    from concourse import bass2jax, mybir as mb

    inputs = {k: np.asarray(v) for k, v in inputs.items()}
    has_sdf_bias = bool(
        any(np.any(inputs[k]) for k in
            ("b_sdf1", "b_sdf2", "b_ki", "b_pka1", "b_pka2"))
    )
    nc = _get_program(has_sdf_bias)
    in_maps = [prep_core_inputs(inputs, c, has_sdf_bias) for c in range(NCORES)]

    bass2jax.install_neuronx_cc_hook()
    partition_name = (
        nc.partition_id_tensor.name if nc.partition_id_tensor else None
    )
    in_names, out_names, out_avals, zero_outs = [], [], [], []
    for alloc in nc.m.functions[0].allocations:
        if not isinstance(alloc, mb.MemoryLocationSet):
            continue
        name = alloc.memorylocations[0].name
        if alloc.kind == "ExternalInput":
            if name != partition_name:
                in_names.append(name)
        elif alloc.kind == "ExternalOutput":
            out_names.append(name)
            shape = tuple(alloc.tensor_shape)
            dtype = mb.dt.np(alloc.dtype)
            out_avals.append(jax.core.ShapedArray(shape, dtype))
            zero_outs.append(np.zeros(shape, dtype))
    n_params = len(in_names)
    n_outs = len(out_avals)
    in_names_all = in_names + out_names
    if partition_name is not None:
        in_names_all = in_names_all + [partition_name]

    def _make_body(nchain):
        def _body(*args):
            ins = list(args[:n_params])
            ybufs = list(args[n_params:])
            pid = (
                [bass2jax.partition_id_tensor()]
                if partition_name is not None
                else []
            )
            for _ in range(nchain):
                outs = bass2jax._bass_exec_p.bind(
                    *ins,
                    *ybufs,
                    *pid,
                    out_avals=tuple(out_avals),
                    in_names=tuple(in_names_all),
                    out_names=tuple(out_names),
                    lowering_input_output_aliases=(),
                    sim_require_finite=True,
                    sim_require_nnan=True,
                    nc=nc,
                )
                ybufs = list(outs)
            return tuple(ybufs)

        return _body

    devices = jax.devices()[:NCORES]
    mesh = Mesh(np.asarray(devices), ("core",))
    in_specs = (PartitionSpec("core"),) * (n_params + n_outs)
    out_specs = (PartitionSpec("core"),) * n_outs

    def _make_sharded(nchain):
        return jax.jit(
            shard_map(
                _make_body(nchain), mesh=mesh, in_specs=in_specs,
                out_specs=out_specs, check_rep=False,
            ),
            keep_unused=True,
        )

    sharded = _make_sharded(1)
    concat_in = [
        np.concatenate([np.asarray(in_maps[c][nm]) for c in range(NCORES)], axis=0)
        for nm in in_names
    ]
    concat_zeros = [
        np.zeros((NCORES * z.shape[0], *z.shape[1:]), z.dtype) for z in zero_outs
    ]
    sharding = jax.sharding.NamedSharding(mesh, PartitionSpec("core"))
    dev_in = [jax.device_put(a, sharding) for a in concat_in]
    dev_zero = [jax.device_put(a, sharding) for a in concat_zeros]

    out_arrs = sharded(*dev_in, *dev_zero)  # warmup/compile
    jax.block_until_ready(out_arrs)

    n_lo = int(os.environ.get("DIMKT_NLO", "16"))
    n_hi = int(os.environ.get("DIMKT_NHI", "80"))

    def best_of(k, nexec):
        best = float("inf")
        for _ in range(k):
            t0 = time.perf_counter()
            os_ = [sharded(*dev_in, *dev_zero) for _ in range(nexec)]
            jax.block_until_ready(os_)
            best = min(best, time.perf_counter() - t0)
        return best

    w1 = best_of(iters, n_lo)
    wn = best_of(iters, n_hi)
    per_exec_ns = int((wn - w1) / (n_hi - n_lo) * 1e9)

    res = [
        {
            nm: np.asarray(out_arrs[i]).reshape(NCORES, *out_avals[i].shape)[c]
            for i, nm in enumerate(out_names)
        }
        for c in range(NCORES)
    ]
    return decode_y(res), per_exec_ns


def run(inputs, **spmd_kwargs):
    """Run on the 8 cores; returns (y [B, T] float32, BassKernelResults)."""
    inputs = {k: np.asarray(v) for k, v in inputs.items()}
    has_sdf_bias = bool(
        any(np.any(inputs[k]) for k in
            ("b_sdf1", "b_sdf2", "b_ki", "b_pka1", "b_pka2"))
    )
    nc = _get_program(has_sdf_bias)
    in_maps = [prep_core_inputs(inputs, c, has_sdf_bias) for c in range(NCORES)]
    res = run_bass_kernel_spmd(nc, in_maps, core_ids=list(range(NCORES)), **spmd_kwargs)
    return decode_y(res.results), res


def kernel(**inputs):
    return run(inputs)[0]


if __name__ == "__main__":
    np.random.seed(0)
    print("building program...")
    import time

    t0 = time.time()
    nc = build_program()
    print("built in %.1fs" % (time.time() - t0))

